# revision 1
# baseline (speedup 1.0000x reference)
"""TRN2 Bass kernel for nn_CML_87969520157217 (retrieval_knn).

scores[u, i] = -||U[u] - I[i]||^2 = 2*U[u]·I[i] - ||I[i]||^2 - ||U[u]||^2

The whole computation folds into ONE matmul with an augmented contraction
dim (K = 64 + 2):
    lhsT[0:64, m] = 2 * u[m, d]     rhs[0:64, n] = item[n, d]
    lhsT[64, m]   = -1              rhs[64, n]   = ||item[n]||^2
    lhsT[65, m]   = -||u[m]||^2     rhs[65, n]   = 1

Sharding: items (and the [256, I] scores) split along the item axis across
8 cores; the 256 looked-up user vectors are replicated. Per core the kernel
is a DMA-bound stream: load rhs slab tiles, matmul into PSUM, copy
PSUM->SBUF (DVE/ACT alternating), DMA the score slab out.
"""

import numpy as np

import concourse.bacc as bacc
import concourse.mybir as mybir
import concourse.tile as tile
from concourse.bass_utils import run_bass_kernel_spmd

N_CORES = 8
N_SCORE = 256
DIM = 64
N_ITEMS = 500000
I_S = N_ITEMS // N_CORES  # 62500 items per core
K_AUG = DIM + 2  # 66

W = 2500  # item columns per in/out DMA tile
SUB = 500  # item columns per matmul / PSUM bank (<=512 for f32)
NSUB = W // SUB  # 5
NW = I_S // W  # 25

_CACHE: dict = {}


def _build_nc():
    nc = bacc.Bacc("TRN2", target_bir_lowering=False, debug=False)
    lhsT = nc.declare_dram_parameter(
        "lhsT", [K_AUG, N_SCORE], mybir.dt.float32, isOutput=False
    )
    rhs = nc.declare_dram_parameter(
        "rhs", [K_AUG, I_S], mybir.dt.float32, isOutput=False
    )
    out = nc.declare_dram_parameter(
        "out", [N_SCORE, I_S], mybir.dt.float32, isOutput=True
    )

    with tile.TileContext(nc) as tc:
        with (
            tc.tile_pool(name="const", bufs=1) as cpool,
            tc.tile_pool(name="rhsp", bufs=3) as rhsp,
            tc.tile_pool(name="outp", bufs=3) as outp,
            tc.tile_pool(name="ps", bufs=8, space="PSUM") as psp,
        ):
            lt = cpool.tile([K_AUG, N_SCORE], mybir.dt.float32)
            nc.sync.dma_start(lt[:], lhsT[:])
            alt = 0
            for w in range(NW):
                rt = rhsp.tile([K_AUG, W], mybir.dt.float32, name="rt")
                nc.sync.dma_start(rt[:], rhs[:, w * W : (w + 1) * W])
                for h in range(2):
                    ot = outp.tile([128, W], mybir.dt.float32, name="ot")
                    for s in range(NSUB):
                        ps = psp.tile([128, SUB], mybir.dt.float32, name="ps")
                        nc.tensor.matmul(
                            ps[:],
                            lt[:, h * 128 : (h + 1) * 128],
                            rt[:, s * SUB : (s + 1) * SUB],
                            start=True,
                            stop=True,
                        )
                        if alt % 2 == 0:
                            nc.vector.tensor_copy(ot[:, s * SUB : (s + 1) * SUB], ps[:])
                        else:
                            nc.scalar.copy(ot[:, s * SUB : (s + 1) * SUB], ps[:])
                        alt += 1
                    nc.sync.dma_start(
                        out[h * 128 : (h + 1) * 128, w * W : (w + 1) * W], ot[:]
                    )
    nc.compile()
    return nc


def _get_nc():
    if "nc" not in _CACHE:
        _CACHE["nc"] = _build_nc()
    return _CACHE["nc"]


def _prep_inputs(score_user_ids, user_embeddings, item_embeddings):
    ids = np.asarray(score_user_ids).astype(np.int64)
    users = np.asarray(user_embeddings, dtype=np.float32)
    items = np.asarray(item_embeddings, dtype=np.float32)

    u = users[ids]  # [256, 64]
    u_sq = np.einsum("md,md->m", u.astype(np.float64), u.astype(np.float64))
    i_sq = np.einsum("nd,nd->n", items.astype(np.float64), items.astype(np.float64))

    lhsT = np.empty((K_AUG, N_SCORE), dtype=np.float32)
    lhsT[0:DIM] = (2.0 * u).T
    lhsT[DIM] = -1.0
    lhsT[DIM + 1] = -u_sq.astype(np.float32)

    in_maps = []
    for c in range(N_CORES):
        sl = slice(c * I_S, (c + 1) * I_S)
        rhs = np.empty((K_AUG, I_S), dtype=np.float32)
        rhs[0:DIM] = items[sl].T
        rhs[DIM] = i_sq[sl].astype(np.float32)
        rhs[DIM + 1] = 1.0
        in_maps.append({"lhsT": lhsT, "rhs": rhs})
    return in_maps


def run(inputs: dict, trace: bool = False):
    """Returns (full_scores[256, 500000] f32, exec_time_ns_or_None)."""
    nc = _get_nc()
    in_maps = _prep_inputs(**inputs)
    res = run_bass_kernel_spmd(nc, in_maps, list(range(N_CORES)), trace=trace)
    scores = np.concatenate([res.results[c]["out"] for c in range(N_CORES)], axis=1)
    return scores, res.exec_time_ns


def kernel(**inputs) -> np.ndarray:
    scores, _ = run(inputs)
    return scores


# revision 2
# speedup vs baseline: 1.1980x; 1.1980x over previous
"""TRN2 Bass kernel for nn_CML_87969520157217 (retrieval_knn).

scores[u, i] = -||U[u] - I[i]||^2 = 2*U[u]·I[i] - ||I[i]||^2 - ||U[u]||^2

The whole computation folds into ONE matmul with an augmented contraction
dim (K = 64 + 2):
    lhsT[0:64, m] = 2 * u[m, d]     rhs[0:64, n] = item[n, d]
    lhsT[64, m]   = -1              rhs[64, n]   = ||item[n]||^2
    lhsT[65, m]   = -||u[m]||^2     rhs[65, n]   = 1

Precision: fp32 matmuls run the PE at quarter rate (2 passes, and the HAM
clock gate never warms for them), so the matmul uses the compensated bf16
split  x·y ≈ xh·yh + xh·yl + xl·yh  accumulated in fp32 PSUM. bf16
products are exact in fp32; only the dropped xl·yl term (~2^-18 relative)
remains: measured ~1.3e-5 relative-to-scale on HW.

Sharding: items (and the [256, I] scores) split along the item axis across
8 cores; the 256 looked-up user vectors are replicated. Per core the kernel
is a DMA-bound stream: load rhs slab tiles, 3-pass matmul into PSUM, copy
PSUM->SBUF (DVE/ACT alternating), DMA the score slab out.
"""

import ml_dtypes
import numpy as np

import concourse.bacc as bacc
import concourse.mybir as mybir
import concourse.tile as tile
from concourse.bass_utils import run_bass_kernel_spmd

N_CORES = 8
N_SCORE = 256
DIM = 64
N_ITEMS = 500000
I_S = N_ITEMS // N_CORES  # 62500 items per core
K_AUG = DIM + 2  # 66

W = 2500  # item columns per out-DMA tile
SUB = 500  # item columns per matmul / PSUM bank (<=512 for f32 PSUM)
NSUB = W // SUB  # 5
NW = I_S // W  # 25

BF16 = mybir.dt.bfloat16
F32 = mybir.dt.float32

_CACHE: dict = {}


def _build_nc():
    nc = bacc.Bacc("TRN2", target_bir_lowering=False, debug=False)
    # lhsT packed [hi | lo] along free dim: [66, 512] bf16
    lhsT = nc.declare_dram_parameter("lhsT", [K_AUG, 2 * N_SCORE], BF16, isOutput=False)
    # rhs packed per W-tile: [66, NW * 2 * W], tile w holds [hi(W) | lo(W)]
    rhs = nc.declare_dram_parameter("rhs", [K_AUG, 2 * I_S], BF16, isOutput=False)
    out = nc.declare_dram_parameter("out", [N_SCORE, I_S], F32, isOutput=True)

    with tile.TileContext(nc) as tc:
        with (
            tc.tile_pool(name="const", bufs=1) as cpool,
            tc.tile_pool(name="rhsp", bufs=3) as rhsp,
            tc.tile_pool(name="outp", bufs=3) as outp,
            tc.tile_pool(name="ps", bufs=8, space="PSUM") as psp,
        ):
            lt = cpool.tile([K_AUG, 2 * N_SCORE], BF16)
            nc.sync.dma_start(lt[:], lhsT[:])
            # weight slices: hi halves then lo halves
            w_hi = [lt[:, 0:128], lt[:, 128:256]]
            w_lo = [lt[:, 256:384], lt[:, 384:512]]
            alt = 0
            for w in range(NW):
                rt = rhsp.tile([K_AUG, 2 * W], BF16, name="rt")
                nc.sync.dma_start(rt[:], rhs[:, w * 2 * W : (w + 1) * 2 * W])
                for h in range(2):
                    ot = outp.tile([128, W], F32, name="ot")
                    for s in range(NSUB):
                        r_hi = rt[:, s * SUB : (s + 1) * SUB]
                        r_lo = rt[:, W + s * SUB : W + (s + 1) * SUB]
                        ps = psp.tile([128, SUB], F32, name="ps")
                        nc.tensor.matmul(ps[:], w_hi[h], r_hi, start=True, stop=False)
                        nc.tensor.matmul(ps[:], w_hi[h], r_lo, start=False, stop=False)
                        nc.tensor.matmul(ps[:], w_lo[h], r_hi, start=False, stop=True)
                        if alt % 2 == 0:
                            nc.vector.tensor_copy(ot[:, s * SUB : (s + 1) * SUB], ps[:])
                        else:
                            nc.scalar.copy(ot[:, s * SUB : (s + 1) * SUB], ps[:])
                        alt += 1
                    nc.sync.dma_start(
                        out[h * 128 : (h + 1) * 128, w * W : (w + 1) * W], ot[:]
                    )
    nc.compile()
    return nc


def _get_nc():
    if "nc" not in _CACHE:
        _CACHE["nc"] = _build_nc()
    return _CACHE["nc"]


def _split_bf16(x: np.ndarray):
    hi = x.astype(ml_dtypes.bfloat16)
    lo = (x - hi.astype(np.float32)).astype(ml_dtypes.bfloat16)
    return hi, lo


def _prep_inputs(score_user_ids, user_embeddings, item_embeddings):
    ids = np.asarray(score_user_ids).astype(np.int64)
    users = np.asarray(user_embeddings, dtype=np.float32)
    items = np.asarray(item_embeddings, dtype=np.float32)

    u = users[ids]  # [256, 64]
    u_sq = np.einsum("md,md->m", u.astype(np.float64), u.astype(np.float64))
    i_sq = np.einsum("nd,nd->n", items.astype(np.float64), items.astype(np.float64))

    lhsT_f32 = np.empty((K_AUG, N_SCORE), dtype=np.float32)
    lhsT_f32[0:DIM] = (2.0 * u).T
    lhsT_f32[DIM] = -1.0
    lhsT_f32[DIM + 1] = -u_sq.astype(np.float32)
    lh, ll = _split_bf16(lhsT_f32)
    lhsT = np.concatenate([lh, ll], axis=1)  # [66, 512] bf16, [hi|lo]

    in_maps = []
    for c in range(N_CORES):
        sl = slice(c * I_S, (c + 1) * I_S)
        rhs_f32 = np.empty((K_AUG, I_S), dtype=np.float32)
        rhs_f32[0:DIM] = items[sl].T
        rhs_f32[DIM] = i_sq[sl].astype(np.float32)
        rhs_f32[DIM + 1] = 1.0
        rh, rl = _split_bf16(rhs_f32)
        # pack per W-tile: [66, NW, 2, W] -> [66, 2*I_S]
        packed = np.stack(
            [rh.reshape(K_AUG, NW, W), rl.reshape(K_AUG, NW, W)], axis=2
        ).reshape(K_AUG, 2 * I_S)
        in_maps.append({"lhsT": lhsT, "rhs": packed})
    return in_maps


def run(inputs: dict, trace: bool = False):
    """Returns (full_scores[256, 500000] f32, exec_time_ns_or_None)."""
    nc = _get_nc()
    in_maps = _prep_inputs(**inputs)
    res = run_bass_kernel_spmd(nc, in_maps, list(range(N_CORES)), trace=trace)
    scores = np.concatenate([res.results[c]["out"] for c in range(N_CORES)], axis=1)
    return scores, res.exec_time_ns


def kernel(**inputs) -> np.ndarray:
    scores, _ = run(inputs)
    return scores


# revision 4
# speedup vs baseline: 2.1635x; 1.8059x over previous
"""TRN2 Bass kernel for nn_CML_87969520157217 (retrieval_knn).

scores[u, i] = -||U[u] - I[i]||^2 = 2*U[u]·I[i] - ||I[i]||^2 - ||U[u]||^2

Decomposition (compensated bf16, fp32 PSUM accumulation; on this platform
the PE runs fp32 matmuls at quarter rate and never engages the HAM clock
boost, so bf16 passes are the fast path):

  With uh/ul = bf16 hi/lo of (2U)^T and Ih/Il = bf16 hi/lo of items^T:
    scores ~= uh·Ih + uh·Il + ul·Ih - i_sq - u_sq      (ul·Il dropped)

  rhs tile T [128, W] per item block:   rows 0:64   = Ih (64 dims)
                                        rows 64:66  = i_sq hi, i_sq lo
                                        rows 66:128 = Il dims 0..61
  MM1 (K=128): lhsT rows = [uh; 0; 0; uh dims 0..61] -> uh·Ih + uh·Il[0:62]
  MM2 (K=66):  lhsT rows = [ul; -1; -1]              -> ul·Ih - i_sq
  u_sq is added as a per-partition fp32 bias during the PSUM->SBUF copy.
  (uh·Il dims 62-63 are dropped: ~1e-5 relative-to-scale error.)

Sharding: items (and the [256, I] scores) split along the item axis across
8 cores; the 256 looked-up user vectors are replicated. Per core the kernel
streams: load rhs tile (ACT ring), 2-pass matmul into PSUM, biased copy
PSUM->SBUF (DVE/ACT alternating), DMA the score slab out (SP ring).
"""

import ml_dtypes
import numpy as np

import concourse.bacc as bacc
import concourse.mybir as mybir
import concourse.tile as tile
from concourse.bass_utils import run_bass_kernel_spmd

N_CORES = 8
N_SCORE = 256
DIM = 64
N_ITEMS = 500000
I_S = N_ITEMS // N_CORES  # 62500 items per core
NLO = 62  # lo-dims carried in the rhs tile (dims 62,63 dropped)

W = 2500  # item columns per out-DMA tile
SUB = 500  # item columns per matmul / PSUM bank (<=512 for f32 PSUM)
NSUB = W // SUB  # 5
NW = I_S // W  # 25

BF16 = mybir.dt.bfloat16
F32 = mybir.dt.float32

_CACHE: dict = {}


def _build_nc():
    nc = bacc.Bacc("TRN2", target_bir_lowering=False, debug=False)
    l1 = nc.declare_dram_parameter("l1", [128, N_SCORE], BF16, isOutput=False)
    l2 = nc.declare_dram_parameter("l2", [66, N_SCORE], BF16, isOutput=False)
    usq = nc.declare_dram_parameter("usq", [128, 2], F32, isOutput=False)
    rhs = nc.declare_dram_parameter("rhs", [128, I_S], BF16, isOutput=False)
    out = nc.declare_dram_parameter("out", [N_SCORE, I_S], F32, isOutput=True)

    with tile.TileContext(nc) as tc:
        with (
            tc.tile_pool(name="const", bufs=1) as cpool,
            tc.tile_pool(name="rhsp", bufs=3) as rhsp,
            tc.tile_pool(name="outp", bufs=3) as outp,
            tc.tile_pool(name="ps", bufs=8, space="PSUM") as psp,
        ):
            tl1 = cpool.tile([128, N_SCORE], BF16)
            tl2 = cpool.tile([66, N_SCORE], BF16)
            tusq = cpool.tile([128, 2], F32)
            nc.sync.dma_start(tl1[:], l1[:])
            nc.sync.dma_start(tl2[:], l2[:])
            nc.sync.dma_start(tusq[:], usq[:])
            alt = 0
            for w in range(NW):
                rt = rhsp.tile([128, W], BF16, name="rt")
                nc.scalar.dma_start(rt[:], rhs[:, w * W : (w + 1) * W])
                for h in range(2):
                    hsl = slice(h * 128, (h + 1) * 128)
                    ot = outp.tile([128, W], F32, name="ot")
                    for s in range(NSUB):
                        ssl = slice(s * SUB, (s + 1) * SUB)
                        ps = psp.tile([128, SUB], F32, name="ps")
                        nc.tensor.matmul(
                            ps[:], tl1[:, hsl], rt[:, ssl], start=True, stop=False
                        )
                        nc.tensor.matmul(
                            ps[:], tl2[:, hsl], rt[0:66, ssl], start=False, stop=True
                        )
                        if alt % 2 == 0:
                            nc.vector.tensor_scalar_add(
                                ot[:, ssl], ps[:], tusq[:, h : h + 1]
                            )
                        else:
                            nc.scalar.activation(
                                ot[:, ssl],
                                ps[:],
                                mybir.ActivationFunctionType.Identity,
                                bias=tusq[:, h : h + 1],
                            )
                        alt += 1
                    nc.sync.dma_start(
                        out[h * 128 : (h + 1) * 128, w * W : (w + 1) * W], ot[:]
                    )
    nc.compile()
    return nc


def _get_nc():
    if "nc" not in _CACHE:
        _CACHE["nc"] = _build_nc()
    return _CACHE["nc"]


def _split_bf16(x: np.ndarray):
    hi = x.astype(ml_dtypes.bfloat16)
    lo = (x - hi.astype(np.float32)).astype(ml_dtypes.bfloat16)
    return hi, lo


def _prep_inputs(score_user_ids, user_embeddings, item_embeddings):
    ids = np.asarray(score_user_ids).astype(np.int64)
    users = np.asarray(user_embeddings, dtype=np.float32)
    items = np.asarray(item_embeddings, dtype=np.float32)

    u = users[ids]  # [256, 64]
    u_sq = np.einsum("md,md->m", u.astype(np.float64), u.astype(np.float64))
    i_sq = np.einsum("nd,nd->n", items.astype(np.float64), items.astype(np.float64))

    uh, ul = _split_bf16((2.0 * u).T)  # [64, 256] each
    ish, isl = _split_bf16(i_sq.astype(np.float32))  # [500000]

    l1 = np.zeros((128, N_SCORE), dtype=ml_dtypes.bfloat16)
    l1[0:DIM] = uh
    l1[DIM + 2 :] = uh[0:NLO]
    l2 = np.empty((66, N_SCORE), dtype=ml_dtypes.bfloat16)
    l2[0:DIM] = ul
    l2[DIM] = -1.0
    l2[DIM + 1] = -1.0
    usq = np.empty((128, 2), dtype=np.float32)
    usq[:, 0] = -u_sq[0:128]
    usq[:, 1] = -u_sq[128:256]

    itemsT = np.ascontiguousarray(items.T)  # [64, 500000]
    ih, il = _split_bf16(itemsT)

    in_maps = []
    for c in range(N_CORES):
        sl = slice(c * I_S, (c + 1) * I_S)
        rhs = np.empty((128, I_S), dtype=ml_dtypes.bfloat16)
        rhs[0:DIM] = ih[:, sl]
        rhs[DIM] = ish[sl]
        rhs[DIM + 1] = isl[sl]
        rhs[DIM + 2 :] = il[0:NLO, sl]
        in_maps.append({"l1": l1, "l2": l2, "usq": usq, "rhs": rhs})
    return in_maps


def run(inputs: dict, trace: bool = False):
    """Returns (full_scores[256, 500000] f32, exec_time_ns_or_None)."""
    nc = _get_nc()
    in_maps = _prep_inputs(**inputs)
    res = run_bass_kernel_spmd(nc, in_maps, list(range(N_CORES)), trace=trace)
    scores = np.concatenate([res.results[c]["out"] for c in range(N_CORES)], axis=1)
    return scores, res.exec_time_ns


def kernel(**inputs) -> np.ndarray:
    scores, _ = run(inputs)
    return scores
